# revision 1
# baseline (speedup 1.0000x reference)
"""Bass/Tile TRN2 kernel for nn_FCMTLSTMFull (CNN + BiLSTM + attention + MoE head).

kernel(**inputs) takes FULL unsharded inputs (as from setup_inputs) and returns
the FULL (8192, 1) float32 output.  Batch is sharded 8 ways (data parallel),
params replicated; one fused Bass program runs SPMD on cores 0-7.
"""

import math
from contextlib import ExitStack

import numpy as np
import ml_dtypes

import concourse.bass as bass
import concourse.mybir as mybir
import concourse.tile as tile
from concourse import bacc
from concourse.bass import ds, ts

F32 = mybir.dt.float32
F32R = mybir.dt.float32r
BF16 = mybir.dt.bfloat16
I32 = mybir.dt.int32
AF = mybir.ActivationFunctionType
ALU = mybir.AluOpType
AX = mybir.AxisListType

BF = ml_dtypes.bfloat16

N_CORES = 8
B_FULL = 8192
IN = 184
H = 128
SEQ = 4
FPS = 46
EPS = 1e-5

SPAN = 16            # samples per conv span
LP = IN + 2          # padded per-sample length (186)
CHW = 2              # samples per conv matmul chunk (N = CHW*IN = 368)
NCV = CHW * IN       # 368


def r32(ap):
    return ap.bitcast(F32R)


def _mm_group(nc, out, pairs):
    n = len(pairs)
    for i, (lhsT, rhs) in enumerate(pairs):
        nc.tensor.matmul(out, lhsT, rhs, start=(i == 0), stop=(i == n - 1))


def build_program(bc):
    nc = bacc.Bacc(None, debug=False)

    spans = bc // SPAN
    ncol = min(512, bc)
    nh = bc // ncol

    di = {}

    def inp(name, shape, dt):
        di[name] = nc.dram_tensor(name, list(shape), dt, kind="ExternalInput")
        return di[name]

    # activations
    inp("xp", (bc, LP), BF16)            # x row-major, zero-padded both ends
    inp("xT", (IN, bc), BF16)            # x transposed (fp proj)
    inp("lab", (1, bc), BF16)            # group labels (as float)
    # conv branch weights (partition dim first everywhere)
    inp("c1wT", (35, 128), BF16)         # conv1 lhsT rows 0-2 and 32-34
    inp("b1f", (128, 1), F32)
    inp("w2T", (128, 3, 2, 128), BF16)  # [i, k, h, o] (bn1 scale folded)
    inp("b2f", (128, 2), F32)
    inp("dwT", (128, 2, 2, 128), BF16)  # [i, ki, mo, o] (bn2 scale folded)
    # temporal branch
    inp("fpTa", (128, IN), BF16)         # fp_w.T rows 0:128
    inp("fpTb", (56, IN), BF16)          # fp_w.T rows 128:184
    inp("fpb", (FPS, SEQ), F32)         # fp_b, col t
    inp("wih0T", (FPS, 2, 512), BF16)   # [i, d, 4H]
    inp("whh0T", (128, 2, 512), BF16)
    inp("b0", (128, 8), F32)            # col = d*4 + g
    inp("wih1T", (128, 2, 2, 512), BF16)  # [i, d, ki, 4H]
    inp("whh1T", (128, 2, 512), BF16)
    inp("b1l", (128, 8), F32)
    inp("awT", (128, 2, 2, 128), BF16)  # [i, ki, mo, o]
    inp("awb", (128, 2), F32)
    inp("av", (128, 2), BF16)           # [i, ki]
    # encoder (bn folded into weights/bias; cnn_db folded into e1b)
    inp("e1Tb", (128, 2, 512), BF16)    # K-chunks 0,1 (spatial part)
    inp("e1Tf", (128, 2, 512), BF16)     # K-chunks 2,3 (context part, f32r)
    inp("e1b", (128, 4), F32)
    inp("e2T", (128, 4, 256), BF16)     # [i, ki, 2*128]
    inp("e2b", (128, 2), F32)
    # experts
    inp("sw1T", (128, 2, 2, 128), BF16)  # [i, e, ki, o]
    inp("sb1", (128, 2), F32)
    inp("sw2T", (128, 2), BF16)          # [i, e]
    inp("sb2", (1, 2), F32)
    inp("lw1T", (128, 2, 2, 2, 128), BF16)  # [i, e, ki, mo, o]
    inp("lb1", (128, 4), F32)               # col = e*2+mo
    inp("lw2T", (128, 2, 2), BF16)          # [i, e, ki]
    inp("lb2", (1, 2), F32)
    out_d = nc.dram_tensor("out", [bc, 1], F32, kind="ExternalOutput")

    with tile.TileContext(nc) as tc:
        with ExitStack() as ctx:
            wp = ctx.enter_context(tc.tile_pool(name="wp", bufs=1))
            pb = ctx.enter_context(tc.tile_pool(name="pb", bufs=1))
            wk = ctx.enter_context(tc.tile_pool(name="wk", bufs=2))
            import os as _os
            _psa = int(_os.environ.get("KPSA", "4"))
            psA = ctx.enter_context(tc.tile_pool(name="psA", bufs=_psa, space="PSUM"))
            psB = ctx.enter_context(tc.tile_pool(name="psB", bufs=8 - _psa, space="PSUM"))

            fpp = ctx.enter_context(tc.tile_pool(name="fpp", bufs=1))
            xTlo = fpp.tile([128, bc], BF16, name="xTlo", tag="xTlo")
            xThi = fpp.tile([56, bc], BF16, name="xThi", tag="xThi")
            fpTa = fpp.tile([128, IN], BF16, name="fpTa", tag="fpTa")
            fpTb = fpp.tile([56, IN], BF16, name="fpTb", tag="fpTb")
            nc.sync.dma_start(xTlo[:], di["xT"][0:128, :])
            nc.sync.dma_start(xThi[:], di["xT"][128:IN, :])
            nc.sync.dma_start(fpTa[:], di["fpTa"][:])
            nc.sync.dma_start(fpTb[:], di["fpTb"][:])

            W = {}
            for name in ("fpb", "wih0T", "whh0T", "b0",
                         "c1wT", "b1f", "w2T", "b2f",
                         "wih1T", "whh1T", "b1l",
                         "awT", "awb", "av", "dwT",
                         "e1Tb", "e1Tf", "e1b", "e2T", "e2b",
                         "sw1T", "sb1", "sw2T", "sb2", "lw1T", "lb1", "lw2T", "lb2"):
                d = di[name]
                t = wp.tile(list(d.shape), d.dtype, name=f"W{name}", tag=f"W{name}")
                nc.sync.dma_start(t[:], d[:])
                W[name] = t

            # ---------------- fp projection ----------------
            xt = [pb.tile([FPS, bc], BF16, name=f"xt{t}", tag=f"xt{t}")
                  for t in range(SEQ)]
            if True:
                for t in range(SEQ):
                    for n in range(nh):
                        cols = ds(n * ncol, ncol)
                        ps = psA.tile([FPS, 512], F32, tag="ps", name=f"fp{t}_{n}")
                        _mm_group(nc, ps[:, 0:ncol], [
                            (fpTa[:, ds(t * FPS, FPS)], xTlo[:, cols]),
                            (fpTb[:, ds(t * FPS, FPS)], xThi[:, cols]),
                        ])
                        nc.scalar.activation(xt[t][:, cols], ps[:, 0:ncol],
                                             AF.Identity,
                                             bias=W["fpb"][:, t:t + 1], scale=1.0)

            # ---------------- BiLSTM ----------------
            h0 = {}
            h1 = {}
            for (lay, hs) in ((0, h0), (1, h1)):
                for t in range(SEQ):
                    for d in range(2):
                        hs[(t, d)] = pb.tile([128, bc], BF16,
                                             name=f"h{lay}_{t}_{d}",
                                             tag=f"h{lay}_{t}_{d}")

            def lstm_dir(lay, hs, dsel):
                for d in (dsel,):
                    h_prev = None
                    c_prev = None
                    for step in range(SEQ):
                        t = step if d == 0 else SEQ - 1 - step
                        c_cur = wk.tile([128, bc], F32, tag="c", bufs=2,
                                        name=f"c{lay}_{d}_{step}")
                        for n in range(nh):
                            cols = ds(n * ncol, ncol)
                            gv = {}
                            for g in range(4):
                                if step == 0 and g == 1:
                                    continue
                                gs = ds(g * 128, 128)
                                ps = psA.tile([128, 512], F32, tag="ps",
                                              name=f"g{lay}_{d}_{step}_{n}_{g}")
                                if lay == 0:
                                    pairs = [(W["wih0T"][:, d, gs], xt[t][:, cols])]
                                else:
                                    pairs = [
                                        (W["wih1T"][:, d, 0, gs], h0[(t, 0)][:, cols]),
                                        (W["wih1T"][:, d, 1, gs], h0[(t, 1)][:, cols]),
                                    ]
                                if step > 0:
                                    pairs.append(
                                        (W[f"whh{lay}T"][:, d, gs], h_prev[:, cols]))
                                _mm_group(nc, ps[:, 0:ncol], pairs)
                                act = AF.Tanh if g == 2 else AF.Sigmoid
                                gt = wk.tile([128, ncol], F32, tag=f"gate{g}",
                                             bufs=2,
                                             name=f"gt{lay}_{d}_{step}_{n}_{g}")
                                bcol = d * 4 + g
                                nc.scalar.activation(
                                    gt[:], ps[:, 0:ncol], act,
                                    bias=W[f"b{lay}" if lay == 0 else "b1l"][:, bcol:bcol + 1],
                                    scale=1.0)
                                gv[g] = gt
                            if step == 0:
                                nc.vector.tensor_mul(c_cur[:, cols], gv[0][:], gv[2][:])
                            else:
                                ig = wk.tile([128, ncol], F32, tag="ig", bufs=2,
                                             name=f"ig{lay}_{d}_{step}_{n}")
                                nc.vector.tensor_mul(ig[:], gv[0][:], gv[2][:])
                                nc.vector.tensor_mul(c_cur[:, cols], gv[1][:],
                                                     c_prev[:, cols])
                                nc.vector.tensor_add(c_cur[:, cols], c_cur[:, cols],
                                                     ig[:])
                            tch = wk.tile([128, ncol], F32, tag="tch", bufs=2,
                                          name=f"tc{lay}_{d}_{step}_{n}")
                            nc.scalar.activation(tch[:], c_cur[:, cols], AF.Tanh)
                            nc.vector.tensor_mul(hs[(t, d)][:, cols], gv[3][:], tch[:])
                        h_prev = hs[(t, d)]
                        c_prev = c_cur

            # ---------------- conv branch (emitted interleaved) ----------
            pool_ = [pb.tile([128, bc], BF16, name=f"pool{hh}", tag=f"pool{hh}")
                     for hh in range(2)]
            cv = ctx.enter_context(tc.tile_pool(name="cv", bufs=2))

            def emit_conv_span(sp):
                s0 = sp * SPAN
                s0 = sp * SPAN
                t3 = cv.tile([35, SPAN * IN], BF16, tag="t3", name=f"t3_{sp}")
                t3v = t3[:].rearrange("p (s l) -> p s l", l=IN)
                # one DMA: 3 shifted windows of each padded sample row
                # (overlapping source reads along the k dim)
                src3 = bass.AP(di["xp"], s0 * LP,
                               [[1, 3], [LP, SPAN], [1, IN]])
                nc.sync.dma_start(t3v[0:3, :, :], src3)
                nc.sync.dma_start(t3v[32:35, :, :], src3)
                h1s = cv.tile([128, SPAN * LP], BF16, tag="h1s",
                              name=f"h1s_{sp}")
                h1v = h1s[:].rearrange("p (s l) -> p s l", l=LP)
                nc.gpsimd.memset(h1v[:, :, 0:1], 0.0)
                nc.gpsimd.memset(h1v[:, :, 185:186], 0.0)
                for c in range(0, SPAN // CHW, 2):
                    # two chunks run concurrently in PE row-groups 0 and 1
                    for ci, pbase in ((c, 0), (c + 1, 32)):
                        ps1 = psA.tile([128, 512], F32, tag="ps",
                                       name=f"c1_{sp}_{ci}")
                        nc.tensor.matmul(
                            ps1[:, 0:NCV],
                            W["c1wT"][pbase:pbase + 3, :],
                            t3[pbase:pbase + 3, ds(ci * NCV, NCV)],
                            start=True, stop=True)
                        nc.scalar.activation(
                            h1v[:, ds(ci * CHW, CHW), 1:185],
                            ps1[:, 0:NCV].rearrange("p (w l) -> p w l", l=IN),
                            AF.Relu, bias=W["b1f"][:, 0:1], scale=1.0)
                for c in range(SPAN // CHW):
                    bs = c * CHW
                    smp = ds(s0 + bs, CHW)
                    rv = h1s[:, ds(bs * LP, CHW * LP)].rearrange(
                        "p (w l) -> p w l", l=LP)
                    for hh in range(2):
                        ps2 = psB.tile([128, 512], F32, tag="psc",
                                       name=f"c2_{sp}_{c}_{hh}")
                        for k in range(3):
                            nc.tensor.matmul(
                                ps2[:, 0:NCV], W["w2T"][:, k, hh, :],
                                rv[:, :, ds(k, IN)],
                                start=(k == 0), stop=(k == 2))
                        pvv = ps2[:, 0:NCV].rearrange(
                            "p (w l) -> p w l", l=IN)
                        _amode = _os.environ.get("KCONVA", "act")
                        act_path = (_amode == "act" or
                                    (_amode == "alt" and c % 2 == 0))
                        if hh == 0 and act_path:
                            tE = wk.tile([128, CHW, IN], BF16, tag="tE",
                                         bufs=3, name=f"tE{sp}_{c}")
                            nc.scalar.activation(
                                tE[:], pvv, AF.Relu,
                                bias=W["b2f"][:, 0:1], scale=1.0)
                            nc.vector.tensor_reduce(
                                pool_[0][:, smp], tE[:], axis=AX.X,
                                op=ALU.max)
                        elif hh == 0:
                            rrA = wk.tile([128, CHW], F32, tag="rrA",
                                          bufs=3, name=f"rrA{sp}_{c}")
                            nc.vector.tensor_reduce(rrA[:], pvv, axis=AX.X,
                                                    op=ALU.max)
                            nc.vector.tensor_scalar(
                                pool_[0][:, smp], rrA[:],
                                W["b2f"][:, 0:1], 0.0,
                                ALU.add, ALU.max)
                        else:
                            rr = wk.tile([128, CHW], F32, tag="rr",
                                         bufs=3, name=f"rr{sp}_{c}")
                            nc.vector.tensor_reduce(rr[:], pvv, axis=AX.X,
                                                    op=ALU.max)
                            nc.vector.tensor_scalar(
                                pool_[1][:, smp], rr[:],
                                W["b2f"][:, 1:2], 0.0,
                                ALU.add, ALU.max)



            q = max(1, spans // 4)
            lstm_dir(0, h0, 0)
            for sp in range(0, q):
                emit_conv_span(sp)
            lstm_dir(0, h0, 1)
            for sp in range(q, 2 * q):
                emit_conv_span(sp)
            lstm_dir(1, h1, 0)
            for sp in range(2 * q, 3 * q):
                emit_conv_span(sp)
            lstm_dir(1, h1, 1)
            for sp in range(3 * q, spans):
                emit_conv_span(sp)

            # ---------------- attention ----------------
            E = [pb.tile([1, bc], BF16, name=f"E{t}", tag=f"E{t}")
                 for t in range(SEQ)]
            for t in range(SEQ):
                u = [wk.tile([128, bc], BF16, tag=f"u{mo}", bufs=2,
                             name=f"u{t}_{mo}") for mo in range(2)]
                for mo in range(2):
                    for n in range(nh):
                        cols = ds(n * ncol, ncol)
                        ps = psA.tile([128, 512], F32, tag="ps",
                                      name=f"at{t}_{mo}_{n}")
                        _mm_group(nc, ps[:, 0:ncol], [
                            (W["awT"][:, 0, mo, :], h1[(t, 0)][:, cols]),
                            (W["awT"][:, 1, mo, :], h1[(t, 1)][:, cols]),
                        ])
                        nc.scalar.activation(u[mo][:, cols], ps[:, 0:ncol], AF.Tanh,
                                             bias=W["awb"][:, mo:mo + 1], scale=1.0)
                for n in range(nh):
                    cols = ds(n * ncol, ncol)
                    ps = psA.tile([1, 512], F32, tag="ps", name=f"sc{t}_{n}")
                    _mm_group(nc, ps[0:1, 0:ncol], [
                        (W["av"][:, 0:1], u[0][:, cols]),
                        (W["av"][:, 1:2], u[1][:, cols]),
                    ])
                    nc.scalar.activation(E[t][0:1, cols], ps[0:1, 0:ncol], AF.Exp)
            SE = wk.tile([1, bc], BF16, tag="se", bufs=1, name="SE")
            nc.vector.tensor_add(SE[:], E[0][:], E[1][:])
            nc.vector.tensor_add(SE[:], SE[:], E[2][:])
            nc.vector.tensor_add(SE[:], SE[:], E[3][:])
            Rr = pb.tile([1, bc], BF16, name="Rr", tag="Rr")
            with nc.allow_low_precision("softmax weights tolerate bf16"):
                nc.vector.reciprocal(Rr[:], SE[:])
            ctxs = [pb.tile([128, bc], F32, name=f"ctx{p}", tag=f"ctx{p}")
                    for p in range(2)]
            ctxb = [pb.tile([128, bc], BF16, name=f"ctxb{p}", tag=f"ctxb{p}")
                    for p in range(2)]
            for s in range(SEQ):
                As = wk.tile([1, bc], BF16, tag="As", bufs=2, name=f"As{s}")
                nc.vector.tensor_mul(As[:], E[s][:], Rr[:])
                AW = wk.tile([128, bc], BF16, tag="AW", bufs=2, name=f"AW{s}")
                nc.gpsimd.partition_broadcast(AW[:], As[0:1, :], channels=128)
                for p in range(2):
                    if s == 0:
                        nc.vector.tensor_mul(ctxs[p][:], h1[(0, p)][:], AW[:])
                    else:
                        cm = wk.tile([128, bc], F32, tag="cm", bufs=2,
                                     name=f"cm{s}_{p}")
                        nc.vector.tensor_mul(cm[:], h1[(s, p)][:], AW[:])
                        dst = ctxb[p] if s == SEQ - 1 else ctxs[p]
                        nc.vector.tensor_add(dst[:], ctxs[p][:], cm[:])

            # ---------------- head ----------------
            zsp = [pb.tile([128, bc], BF16, name=f"zsp{mo}", tag=f"zsp{mo}")
                   for mo in range(2)]
            for mo in range(2):
                for n in range(nh):
                    cols = ds(n * ncol, ncol)
                    ps = psA.tile([128, 512], F32, tag="ps", name=f"sp{mo}_{n}")
                    _mm_group(nc, ps[:, 0:ncol], [
                        (W["dwT"][:, 0, mo, :], pool_[0][:, cols]),
                        (W["dwT"][:, 1, mo, :], pool_[1][:, cols]),
                    ])
                    nc.scalar.activation(zsp[mo][:, cols], ps[:, 0:ncol], AF.Copy)
            z2 = [pb.tile([128, bc], BF16, name=f"z2_{mo}", tag=f"z2_{mo}")
                  for mo in range(4)]
            for mo in range(4):
                ms = ds(mo * 128, 128)
                for n in range(nh):
                    cols = ds(n * ncol, ncol)
                    ps = psA.tile([128, 512], F32, tag="ps", name=f"e1_{mo}_{n}")
                    _mm_group(nc, ps[:, 0:ncol], [
                        (W["e1Tb"][:, 0, ms], zsp[0][:, cols]),
                        (W["e1Tb"][:, 1, ms], zsp[1][:, cols]),
                        (W["e1Tf"][:, 0, ms], ctxb[0][:, cols]),
                        (W["e1Tf"][:, 1, ms], ctxb[1][:, cols]),
                    ])
                    nc.scalar.activation(z2[mo][:, cols], ps[:, 0:ncol], AF.Relu,
                                         bias=W["e1b"][:, mo:mo + 1], scale=1.0)
            enc = [pb.tile([128, bc], BF16, name=f"enc{mo}", tag=f"enc{mo}")
                   for mo in range(2)]
            for mo in range(2):
                ms = ds(mo * 128, 128)
                for n in range(nh):
                    cols = ds(n * ncol, ncol)
                    ps = psA.tile([128, 512], F32, tag="ps", name=f"e2_{mo}_{n}")
                    _mm_group(nc, ps[:, 0:ncol],
                              [(W["e2T"][:, ki, ms], z2[ki][:, cols])
                               for ki in range(4)])
                    nc.scalar.activation(enc[mo][:, cols], ps[:, 0:ncol], AF.Relu,
                                         bias=W["e2b"][:, mo:mo + 1], scale=1.0)

            # experts + routed select
            Lf = pb.tile([1, bc], BF16, name="Lf", tag="Lf")
            nc.sync.dma_start(Lf[:], di["lab"][:])
            pred = pb.tile([1, bc], F32, name="pred", tag="pred")
            for n in range(nh):
                cols = ds(n * ncol, ncol)
                for e in range(4):
                    po = psA.tile([1, 512], F32, tag="ps", name=f"xo{e}_{n}")
                    if e < 2:
                        hhout = wk.tile([128, ncol], BF16, tag="hh", bufs=2,
                                        name=f"hhS{e}_{n}")
                        ph = psA.tile([128, 512], F32, tag="ps", name=f"xh{e}_{n}")
                        _mm_group(nc, ph[:, 0:ncol],
                                  [(W["sw1T"][:, e, ki, :], enc[ki][:, cols])
                                   for ki in range(2)])
                        nc.scalar.activation(hhout[:], ph[:, 0:ncol], AF.Relu,
                                             bias=W["sb1"][:, e:e + 1], scale=1.0)
                        nc.tensor.matmul(po[0:1, 0:ncol], W["sw2T"][:, e:e + 1],
                                         hhout[:], start=True, stop=True)
                        b2ap = W["sb2"][0:1, e:e + 1]
                    else:
                        el = e - 2
                        for mo in range(2):
                            hhout = wk.tile([128, ncol], BF16, tag="hh", bufs=2,
                                            name=f"hhL{el}_{mo}_{n}")
                            ph = psA.tile([128, 512], F32, tag="ps",
                                          name=f"xhL{el}_{mo}_{n}")
                            _mm_group(nc, ph[:, 0:ncol],
                                      [(W["lw1T"][:, el, ki, mo, :],
                                        enc[ki][:, cols]) for ki in range(2)])
                            bcol = el * 2 + mo
                            nc.scalar.activation(hhout[:], ph[:, 0:ncol], AF.Relu,
                                                 bias=W["lb1"][:, bcol:bcol + 1],
                                                 scale=1.0)
                            nc.tensor.matmul(po[0:1, 0:ncol],
                                             W["lw2T"][:, el, mo:mo + 1], hhout[:],
                                             start=(mo == 0), stop=(mo == 1))
                        b2ap = W["lb2"][0:1, el:el + 1]
                    oe = wk.tile([1, ncol], F32, tag="oe", bufs=1,
                                 name=f"oe{e}_{n}")
                    nc.scalar.activation(oe[:], po[0:1, 0:ncol], AF.Identity,
                                         bias=b2ap, scale=1.0)
                    mk = wk.tile([1, ncol], F32, tag="mk", bufs=1,
                                 name=f"mk{e}_{n}")
                    nc.vector.tensor_scalar(mk[:], Lf[0:1, cols], float(e), None,
                                            ALU.is_equal)
                    if e == 0:
                        nc.vector.tensor_mul(pred[0:1, cols], oe[:], mk[:])
                    else:
                        pm = wk.tile([1, ncol], F32, tag="pm", bufs=1,
                                     name=f"pm{e}_{n}")
                        nc.vector.tensor_mul(pm[:], oe[:], mk[:])
                        nc.vector.tensor_add(pred[0:1, cols], pred[0:1, cols],
                                             pm[:])
            nc.sync.dma_start(out_d[:].rearrange("b one -> one b"), pred[:])

    nc.compile()
    return nc


_PROG_CACHE = {}


def _get_program(bc):
    if bc not in _PROG_CACHE:
        _PROG_CACHE[bc] = build_program(bc)
    return _PROG_CACHE[bc]



def _pack_c1(conv1_w):
    w = np.ascontiguousarray(conv1_w[:, 0, :].T)  # (3, 128)
    out = np.zeros((35, 128), np.float32)
    out[0:3] = w
    out[32:35] = w
    return out.astype(BF)

def prep_arrays(inputs, n_cores=N_CORES):
    """Host-side weight fusion/packing.  Returns (shared params dict,
    per-core activation dicts)."""
    f32 = np.float32
    gi = {k: np.asarray(v) for k, v in inputs.items()}
    x = gi["x"].astype(f32)
    lab = gi["group_labels"].astype(np.int32)
    B = x.shape[0]
    bc = B // n_cores

    s = f32(1.0 / math.sqrt(1.0 + EPS))
    g1 = gi["bn1_g"] * s                      # (128,)
    b1f = gi["conv1_b"] + gi["bn1_b"] / g1
    w2 = gi["conv2_w"] * g1[None, :, None]    # (256,128,3)
    # w2T [i, k, h, o]
    w2T = np.ascontiguousarray(np.transpose(w2.reshape(2, 128, 128, 3),
                                            (2, 3, 0, 1))).astype(BF)
    g2 = gi["bn2_g"] * s
    b2f = (gi["conv2_b"] + gi["bn2_b"] / g2).reshape(2, 128).T  # (128,2)
    dw = gi["cnn_dw"] * g2[None, :]           # (256,256) out,in
    # dwT [i, ki, mo, o] = dw[mo*128+o, ki*128+i]
    dwT = np.ascontiguousarray(
        np.transpose(dw.reshape(2, 128, 2, 128), (3, 2, 0, 1))).astype(BF)

    fpT = gi["fp_w"].T.astype(f32)            # (184,184) [i, o]
    fpb = np.ascontiguousarray(gi["fp_b"].reshape(SEQ, FPS).T).astype(f32)

    def pack_T(w):  # (4H, K) -> (K, 4H)
        return np.ascontiguousarray(w.T)

    wih0T = np.stack([pack_T(gi["Wih0"][d]) for d in range(2)], axis=1)  # (46,2,512)
    whh0T = np.stack([pack_T(gi["Whh0"][d]) for d in range(2)], axis=1)  # (128,2,512)
    b0 = np.concatenate([gi["bih0"][d] + gi["bhh0"][d] for d in range(2)])
    b0 = np.ascontiguousarray(b0.reshape(8, 128).T).astype(f32)          # (128,8)
    wih1 = np.stack([pack_T(gi["Wih1"][d]) for d in range(2)], axis=1)   # (256,2,512)
    wih1T = np.ascontiguousarray(
        wih1.reshape(2, 128, 2, 512).transpose(1, 2, 0, 3))              # (128,2,2,512)
    whh1T = np.stack([pack_T(gi["Whh1"][d]) for d in range(2)], axis=1)
    b1l = np.concatenate([gi["bih1"][d] + gi["bhh1"][d] for d in range(2)])
    b1l = np.ascontiguousarray(b1l.reshape(8, 128).T).astype(f32)

    awT = np.ascontiguousarray(
        gi["attW_w"].T.reshape(2, 128, 2, 128).transpose(1, 0, 2, 3)).astype(BF)
    awb = np.ascontiguousarray(gi["attW_b"].reshape(2, 128).T).astype(f32)
    av = np.ascontiguousarray(gi["attv"].reshape(2, 128).T).astype(BF)

    e1b_fold = gi["enc1_b"] + gi["enc1_w"][:, :256] @ gi["cnn_db"]
    es1 = np.ones(512, f32) * s * gi["bne1_g"]
    e1w = gi["enc1_w"] * es1[:, None]          # (512,512)
    e1b = (e1b_fold * es1 + gi["bne1_b"]).reshape(4, 128).T.astype(f32)
    e1T = np.ascontiguousarray(e1w.T.reshape(4, 128, 512))  # [ki][i, 512]
    e1Tb = np.ascontiguousarray(e1T[0:2].transpose(1, 0, 2)).astype(BF)
    e1Tf = np.ascontiguousarray(e1T[2:4].transpose(1, 0, 2)).astype(BF)
    es2 = np.ones(256, f32) * s * gi["bne2_g"]
    e2w = gi["enc2_w"] * es2[:, None]          # (256,512)
    e2b = (gi["enc2_b"] * es2 + gi["bne2_b"]).reshape(2, 128).T.astype(f32)
    e2T = np.ascontiguousarray(
        e2w.T.reshape(4, 128, 256).transpose(1, 0, 2)).astype(BF)

    sw1T = np.ascontiguousarray(
        np.stack([gi["decS_w1"][e].T.reshape(2, 128, 128) for e in range(2)],
                 axis=0).transpose(2, 0, 1, 3)).astype(BF)  # (128,e,ki,128)
    sb1 = np.ascontiguousarray(
        np.stack([gi["decS_b1"][e] for e in range(2)], axis=1)).astype(f32)
    sw2T = np.ascontiguousarray(
        np.stack([gi["decS_w2"][e, 0] for e in range(2)], axis=1)).astype(BF)
    sb2 = gi["decS_b2"].reshape(1, 2).astype(f32)
    lw1T = np.ascontiguousarray(
        np.stack([gi["decL_w1"][e].T.reshape(2, 128, 2, 128) for e in range(2)],
                 axis=0).transpose(2, 0, 1, 3, 4)).astype(BF)  # (128,e,ki,mo,128)
    lb1 = np.ascontiguousarray(
        np.stack([gi["decL_b1"][e].reshape(2, 128) for e in range(2)],
                 axis=0).reshape(4, 128).T).astype(f32)  # col = e*2+mo
    lw2T = np.ascontiguousarray(
        np.stack([gi["decL_w2"][e, 0].reshape(2, 128) for e in range(2)],
                 axis=0).transpose(2, 0, 1)).astype(BF)  # (128,e,ki)
    lb2 = gi["decL_b2"].reshape(1, 2).astype(f32)

    shared = dict(
        c1wT=_pack_c1(gi["conv1_w"]),
        b1f=b1f.reshape(128, 1).astype(f32),
        w2T=w2T, b2f=np.ascontiguousarray(b2f).astype(f32), dwT=dwT,
        fpTa=np.ascontiguousarray(fpT[0:128]).astype(BF),
        fpTb=np.ascontiguousarray(fpT[128:IN]).astype(BF),
        fpb=fpb,
        wih0T=wih0T.astype(BF), whh0T=whh0T.astype(BF), b0=b0,
        wih1T=wih1T.astype(BF), whh1T=whh1T.astype(BF), b1l=b1l,
        awT=awT, awb=awb, av=av,
        e1Tb=e1Tb, e1Tf=e1Tf, e1b=np.ascontiguousarray(e1b),
        e2T=e2T, e2b=np.ascontiguousarray(e2b),
        sw1T=sw1T, sb1=sb1, sw2T=sw2T, sb2=sb2,
        lw1T=lw1T, lb1=lb1, lw2T=lw2T, lb2=lb2,
    )
    per_core = []
    for c in range(n_cores):
        sl = slice(c * bc, (c + 1) * bc)
        per_core.append(dict(
            xp=np.ascontiguousarray(
                np.pad(x[sl], ((0, 0), (1, 1)))).astype(BF),
            xT=np.ascontiguousarray(x[sl].T).astype(BF),
            lab=np.ascontiguousarray(lab[sl].reshape(1, bc).astype(BF)),
        ))
    return shared, per_core, bc


def kernel(**inputs) -> np.ndarray:
    from concourse.bass_utils import run_bass_kernel_spmd

    shared, per_core, bc = prep_arrays(inputs)
    nc = _get_program(bc)
    in_maps = [dict(shared, **pc) for pc in per_core]
    res = run_bass_kernel_spmd(nc, in_maps, core_ids=list(range(N_CORES)))
    out = np.concatenate([res.results[c]["out"] for c in range(N_CORES)], axis=0)
    return out.astype(np.float32)

